# revision 5
# baseline (speedup 1.0000x reference)
"""Trainium2 Bass kernel for nn_ContextualViewModel_48833778155979.

Computation (see reference):
    station_feats = x[sx, sy]            # (K, F) gather -- on host (the
                                         # sharding hint says to replicate it)
    y = station_feats @ W                # (K, F) tiny matmul -- on device
    res[h, w, :] = sum_k d[h, w, k] * y[k, :]   # big (H*W, K) @ (K, F) matmul

Sharding: H axis split across 8 cores (48 rows each -> 18432 grid cells/core).
Per core the big matmul is (18432, 256) @ (256, 256).

The kernel is DMA-roofline bound (HBM ~358 GB/s/core), so the wire format
is minimized:
  - d in [0,1) is quantized host-side to uint8 (q = round(d*255), so
    d ~= q/255 exactly -- no zero-point correction needed) and laid out
    k-major with an 8-way row interleave inside every 1024-row block
    (row blk*1024 + 8p + q stored at column blk*1024 + q*128 + p).
    GpSimd + DVE cast the u8 slabs to fp16 on device; y is pre-scaled by
    1/255 once, so the matmul output needs no fixup. Input: 4.7 MiB/core.
  - The PE consumes the casted 128x128 chunks as the stationary operand
    (no PE transposes); y (fp16, k-major) is the moving operand; PSUM
    accumulates the two 128-wide k chunks in fp32.
  - Each output partition owns 8 consecutive DRAM rows, so every
    1024-row block stores as one DMA with a single contiguous 4 KiB burst
    per partition. Output: 9 MiB/core as fp16.
  - PSUM drains (fp32 -> fp16 casts) are split between DVE and ScalarE;
    store-DMA descriptor gen lives on ScalarE, load-side on SyncE, so no
    engine's semaphore wait can stall another's DMA issue.
  - 8 junk warmup matmuls run while the first d slab streams in, lifting
    the PE HAM clock throttle (1.2 -> 2.4 GHz) before the real work.

PE per core: 288 matmuls x 256 moving rows ~= 31 us warm, under the
~40 us of DMA. Accuracy: u8 input quantization + fp16 wire, fp32
accumulation: end-to-end rel err ~2e-3 (gate 1e-2).
"""

import sys

sys.path.insert(0, "/opt/trn_rl_repo")

from contextlib import ExitStack

import numpy as np

import concourse.bacc as bacc
import concourse.mybir as mybir
import concourse.tile as tile
from concourse.bass_utils import run_bass_kernel_spmd

H, WG, F = 384, 384, 256
K = 256
NCORES = 8
HS = H // NCORES          # 48 grid rows per core
ROWS = HS * WG            # 18432 cells per core
SLAB = 2048               # rows per input DMA slab (0.5 MiB u8)
NSLAB = ROWS // SLAB      # 9
BLK = 1024                # rows per output DMA block / interleave group

F16 = mybir.dt.float16
F32 = mybir.dt.float32
U8 = mybir.dt.uint8

_cache: dict = {}
last_results = None  # BassKernelResults of the most recent kernel() call


def _build_program():
    key = "nc"
    if key in _cache:
        return _cache[key]

    nc = bacc.Bacc(
        "TRN2", target_bir_lowering=False, debug=False, num_devices=NCORES
    )

    # d_q: per-core shard of d, uint8-quantized, k-major with 8-way row
    # interleave (see module docstring):
    #   d_q[k, blk*1024 + q*128 + p] = round(255 * d_shard[blk*1024 + 8p + q, k])
    dq_ext = nc.dram_tensor("d_q", [K, ROWS], U8, kind="ExternalInput").ap()
    # station_t: gathered station features, transposed to (F_contract, K)
    stT_ext = nc.dram_tensor("station_t", [F, K], F16, kind="ExternalInput").ap()
    w_ext = nc.dram_tensor("w_mat", [F, F], F16, kind="ExternalInput").ap()
    out_ext = nc.dram_tensor("out_shard", [ROWS, F], F16, kind="ExternalOutput").ap()

    with tile.TileContext(nc) as tc, ExitStack() as ctx:
        const = ctx.enter_context(tc.tile_pool(name="const", bufs=1))
        d8pool = ctx.enter_context(tc.tile_pool(name="din8", bufs=3))
        dfpool = ctx.enter_context(tc.tile_pool(name="din16", bufs=3))
        opool = ctx.enter_context(tc.tile_pool(name="dout", bufs=4))
        mpsum = ctx.enter_context(tc.tile_pool(name="mpsum", bufs=6, space="PSUM"))
        ypsum = ctx.enter_context(tc.tile_pool(name="ypsum", bufs=1, space="PSUM"))
        wpsum = ctx.enter_context(tc.tile_pool(name="wpsum", bufs=1, space="PSUM"))

        # --- constants (scalar queue: overlaps the first d slab on sync) ---
        stT = const.tile([128, 2, K], F16)
        nc.scalar.dma_start(
            stT[:, :, :], stT_ext.rearrange("(cc cp) k -> cp cc k", cc=2)
        )
        w_sb = const.tile([128, 2, F], F16)
        nc.scalar.dma_start(
            w_sb[:, :, :], w_ext.rearrange("(cc cp) f -> cp cc f", cc=2)
        )

        # --- PE warmup -----------------------------------------------------
        # ~3.5 us of junk matmuls (result never read) while the first d slab
        # streams in: the HAM clock gate needs ~3.4 us of sustained PE
        # activity to lift the idle throttle (1.2 GHz -> 2.4 GHz).
        warm = wpsum.tile([128, 2, F], F32, tag="warm")
        for _ in range(8):
            nc.tensor.matmul(
                warm[:, :, :],
                stT[:, 0, 0:128],
                w_sb[:, :, :],
                start=True,
                stop=True,
            )

        # --- y = (station_feats @ W) / 255, k-major fp16 -------------------
        # yps is one full PSUM bank; each 128-wide k chunk is its own
        # accumulation group in one half of the bank. The 1/255 scale folds
        # the u8 dequantization of d into y.
        y_sb = const.tile([128, 2, F], F16)
        yps = ypsum.tile([128, 2, F], F32, tag="yps")
        for kc in range(2):
            for cc in range(2):
                nc.tensor.matmul(
                    yps[:, kc, :],
                    stT[:, cc, kc * 128 : (kc + 1) * 128],
                    w_sb[:, cc, :],
                    start=(cc == 0),
                    stop=(cc == 1),
                )
        nc.vector.tensor_scalar_mul(y_sb[:, :, :], yps[:, :, :], 1.0 / 255.0)

        # --- main loop: out = d @ y ---------------------------------------
        for s in range(NSLAB):
            din8 = d8pool.tile([128, 2, SLAB], U8, tag="din8")
            nc.sync.dma_start(
                din8[:, :, :],
                dq_ext[:, s * SLAB : (s + 1) * SLAB].rearrange(
                    "(kc kp) r -> kp kc r", kc=2
                ),
            )
            # u8 -> fp16 dequant-cast, split by k chunk across GpSimd / DVE,
            # halved by column so the first block's matmuls start sooner.
            din = dfpool.tile([128, 2, SLAB], F16, tag="din16")
            for half in range(2):
                cs = half * BLK
                nc.gpsimd.tensor_copy(
                    din[:, 0, cs : cs + BLK], din8[:, 0, cs : cs + BLK]
                )
                nc.vector.tensor_copy(
                    din[:, 1, cs : cs + BLK], din8[:, 1, cs : cs + BLK]
                )
            for b in range(SLAB // BLK):
                # dout dims: [p, pr, qq, f] -- DRAM row = 8p + 2*pr + qq
                dout = opool.tile([128, 4, 2, F], F16, tag="dout")
                for pr in range(4):
                    po = mpsum.tile([128, 2, F], F32, tag="po")
                    for qq in range(2):
                        c0 = b * BLK + (pr * 2 + qq) * 128
                        for kc in range(2):
                            nc.tensor.matmul(
                                po[:, qq, :],
                                din[:, kc, c0 : c0 + 128],
                                y_sb[:, kc, :],
                                start=(kc == 0),
                                stop=(kc == 1),
                            )
                    if pr < 2:
                        nc.vector.tensor_copy(dout[:, pr, :, :], po[:, :, :])
                    else:
                        nc.scalar.copy(dout[:, pr, :, :], po[:, :, :])
                nc.scalar.dma_start(
                    out_ext[
                        (s * 2 + b) * BLK : (s * 2 + b + 1) * BLK, :
                    ].rearrange("(p pr qq) f -> p pr qq f", p=128, pr=4),
                    dout[:, :, :, :],
                )

    nc.compile()
    _cache[key] = nc
    return nc


def kernel(x, d, W, sx, sy):
    x = np.asarray(x, dtype=np.float32)
    d = np.asarray(d, dtype=np.float32)
    W = np.asarray(W, dtype=np.float32)
    sx = np.asarray(sx, dtype=np.int32)
    sy = np.asarray(sy, dtype=np.int32)

    # Host-side shard prep, per the sharding strategy: gather the K station
    # feature vectors once (replicated to all cores), pre-transpose the
    # station features and each core's d shard to contraction-major (with the
    # 8-way row interleave the store DMA layout expects), and quantize the
    # wire tensors (d to uint8, the rest to fp16).
    station_t = np.ascontiguousarray(x[sx, sy].T, dtype=np.float16)
    w16 = W.astype(np.float16)

    nc = _build_program()

    nb = ROWS // BLK
    in_maps = []
    for c in range(NCORES):
        d_sh = d[c * HS : (c + 1) * HS].reshape(ROWS, K)
        q8 = np.rint(d_sh * 255.0).astype(np.uint8)
        # [blk, p, q, k] -> [k, blk, q, p]:
        #   d_q[k, blk*1024 + q*128 + p] = q8[blk*1024 + 8p + q, k]
        d_q = np.ascontiguousarray(
            q8.reshape(nb, 128, 8, K).transpose(3, 0, 2, 1)
        ).reshape(K, ROWS)
        in_maps.append(
            {
                "d_q": d_q,
                "station_t": station_t,
                "w_mat": w16,
            }
        )

    res = run_bass_kernel_spmd(nc, in_maps, list(range(NCORES)))
    global last_results
    last_results = res
    out = np.concatenate(
        [
            r["out_shard"].astype(np.float32).reshape(HS, WG, F)
            for r in res.results
        ],
        axis=0,
    )
    return out


if __name__ == "__main__":
    rng = np.random.default_rng(0)
    x = rng.standard_normal((H, WG, F), dtype=np.float32)
    d = rng.random((H, WG, K), dtype=np.float32)
    W = rng.standard_normal((K, F), dtype=np.float32) / np.sqrt(F)
    sx = rng.integers(0, H, size=(K,)).astype(np.int32)
    sy = rng.integers(0, WG, size=(K,)).astype(np.int32)
    out = kernel(x, d, W, sx, sy)
    y = x[sx, sy].astype(np.float64) @ W.astype(np.float64)
    exp = d.reshape(-1, K).astype(np.float64) @ y
    exp = exp.reshape(H, WG, F)
    err = np.linalg.norm(out - exp) / np.linalg.norm(exp)
    print("rel err:", err)


# revision 6
# speedup vs baseline: 1.2616x; 1.2616x over previous
"""Trainium2 Bass kernel for nn_ContextualViewModel_48833778155979.

Computation (see reference):
    station_feats = x[sx, sy]            # (K, F) gather -- on host (the
                                         # sharding hint says to replicate it)
    y = station_feats @ W                # (K, F) tiny matmul -- on device
    res[h, w, :] = sum_k d[h, w, k] * y[k, :]   # big (H*W, K) @ (K, F) matmul

Sharding: H axis split across 8 cores (48 rows each -> 18432 grid cells/core).
Per core the big matmul is (18432, 256) @ (256, 256).

The kernel is DMA-roofline bound (HBM ~358 GB/s/core): mandatory traffic
is d in + out, moved as fp16 (9 + 9 MiB ~= 53 us; fp32 would be ~105 us).
(uint8 input was tried and reverted: GpSimd/DVE convert u8->fp16 at only
~0.3 elem/lane/cycle, costing more than the DMA it saves.)

  - d is laid out k-major during host-side shard prep, with an 8-way row
    interleave inside every 1024-row block (row blk*1024 + 8p + q stored
    at column blk*1024 + q*128 + p). The 128x128 stationary chunks DMA
    straight into SBUF (no PE transposes), input bursts are 4 KiB
    contiguous per partition, and each output partition owns 8
    consecutive DRAM rows so every 1024-row block stores as one DMA with
    a single contiguous 4 KiB burst per partition.
  - y (fp16, k-major) is the moving operand (256 wide); PSUM accumulates
    the two 128-wide k chunks in fp32. One PSUM bank = two 128-row
    output subtiles, drained by a single 512-elem cast to fp16.
  - PSUM drains are split 2/2 between DVE and ScalarE (PSUM has one DVE
    read port, so a single engine would be on the critical path);
    store-DMA descriptor gen lives on ScalarE, load-side on SyncE, and
    the constants load on ScalarE so the head overlaps.
  - 8 junk warmup matmuls run while the first d slab streams in, lifting
    the PE HAM clock throttle (1.2 -> 2.4 GHz) before the real work.

PE per core: 288 matmuls x 256 moving rows ~= 31 us warm, hidden under
~53 us of DMA. Accuracy: fp16 wire quantization, fp32 accumulation;
end-to-end rel err ~5e-4 (gate 1e-2).
"""

import sys

sys.path.insert(0, "/opt/trn_rl_repo")

from contextlib import ExitStack

import numpy as np

import concourse.bacc as bacc
import concourse.mybir as mybir
import concourse.tile as tile
from concourse.bass_utils import run_bass_kernel_spmd

H, WG, F = 384, 384, 256
K = 256
NCORES = 8
HS = H // NCORES          # 48 grid rows per core
ROWS = HS * WG            # 18432 cells per core
SLAB = 2048               # rows per input DMA slab (1 MiB fp16)
NSLAB = ROWS // SLAB      # 9
BLK = 1024                # rows per output DMA block / interleave group

F16 = mybir.dt.float16
F32 = mybir.dt.float32

_cache: dict = {}
last_results = None  # BassKernelResults of the most recent kernel() call


def _build_program():
    key = "nc"
    if key in _cache:
        return _cache[key]

    nc = bacc.Bacc(
        "TRN2", target_bir_lowering=False, debug=False, num_devices=NCORES
    )

    # d_t: per-core shard of d, k-major with 8-way row interleave (see
    # module docstring):
    #   d_t[k, blk*1024 + q*128 + p] = d_shard[blk*1024 + 8p + q, k]
    dt_ext = nc.dram_tensor("d_t", [K, ROWS], F16, kind="ExternalInput").ap()
    # station_t: gathered station features, transposed to (F_contract, K)
    stT_ext = nc.dram_tensor("station_t", [F, K], F16, kind="ExternalInput").ap()
    w_ext = nc.dram_tensor("w_mat", [F, F], F16, kind="ExternalInput").ap()
    out_ext = nc.dram_tensor("out_shard", [ROWS, F], F16, kind="ExternalOutput").ap()

    with tile.TileContext(nc) as tc, ExitStack() as ctx:
        const = ctx.enter_context(tc.tile_pool(name="const", bufs=1))
        dpool = ctx.enter_context(tc.tile_pool(name="din", bufs=3))
        opool = ctx.enter_context(tc.tile_pool(name="dout", bufs=4))
        mpsum = ctx.enter_context(tc.tile_pool(name="mpsum", bufs=6, space="PSUM"))
        ypsum = ctx.enter_context(tc.tile_pool(name="ypsum", bufs=1, space="PSUM"))
        wpsum = ctx.enter_context(tc.tile_pool(name="wpsum", bufs=1, space="PSUM"))

        # --- constants (scalar queue: overlaps the first d slab on sync) ---
        stT = const.tile([128, 2, K], F16)
        nc.scalar.dma_start(
            stT[:, :, :], stT_ext.rearrange("(cc cp) k -> cp cc k", cc=2)
        )
        w_sb = const.tile([128, 2, F], F16)
        nc.scalar.dma_start(
            w_sb[:, :, :], w_ext.rearrange("(cc cp) f -> cp cc f", cc=2)
        )

        # --- PE warmup -----------------------------------------------------
        # ~3.5 us of junk matmuls (result never read) while the first d slab
        # streams in: the HAM clock gate needs ~3.4 us of sustained PE
        # activity to lift the idle throttle (1.2 GHz -> 2.4 GHz).
        warm = wpsum.tile([128, 2, F], F32, tag="warm")
        for _ in range(8):
            nc.tensor.matmul(
                warm[:, :, :],
                stT[:, 0, 0:128],
                w_sb[:, :, :],
                start=True,
                stop=True,
            )

        # --- y = station_feats @ W, k-major in SBUF as fp16 ----------------
        # yps is one full PSUM bank; each 128-wide k chunk is its own
        # accumulation group in one half of the bank.
        y_sb = const.tile([128, 2, F], F16)
        yps = ypsum.tile([128, 2, F], F32, tag="yps")
        for kc in range(2):
            for cc in range(2):
                nc.tensor.matmul(
                    yps[:, kc, :],
                    stT[:, cc, kc * 128 : (kc + 1) * 128],
                    w_sb[:, cc, :],
                    start=(cc == 0),
                    stop=(cc == 1),
                )
        nc.vector.tensor_copy(y_sb[:, :, :], yps[:, :, :])

        # --- main loop: out = d @ y ---------------------------------------
        for s in range(NSLAB):
            din = dpool.tile([128, 2, SLAB], F16, tag="din")
            nc.sync.dma_start(
                din[:, :, :],
                dt_ext[:, s * SLAB : (s + 1) * SLAB].rearrange(
                    "(kc kp) r -> kp kc r", kc=2
                ),
            )
            for b in range(SLAB // BLK):
                # dout dims: [p, pr, qq, f] -- DRAM row = 8p + 2*pr + qq
                dout = opool.tile([128, 4, 2, F], F16, tag="dout")
                for pr in range(4):
                    po = mpsum.tile([128, 2, F], F32, tag="po")
                    for qq in range(2):
                        c0 = b * BLK + (pr * 2 + qq) * 128
                        for kc in range(2):
                            nc.tensor.matmul(
                                po[:, qq, :],
                                din[:, kc, c0 : c0 + 128],
                                y_sb[:, kc, :],
                                start=(kc == 0),
                                stop=(kc == 1),
                            )
                    if pr < 2:
                        nc.vector.tensor_copy(dout[:, pr, :, :], po[:, :, :])
                    else:
                        nc.scalar.copy(dout[:, pr, :, :], po[:, :, :])
                nc.scalar.dma_start(
                    out_ext[
                        (s * 2 + b) * BLK : (s * 2 + b + 1) * BLK, :
                    ].rearrange("(p pr qq) f -> p pr qq f", p=128, pr=4),
                    dout[:, :, :, :],
                )

    nc.compile()
    _cache[key] = nc
    return nc


def kernel(x, d, W, sx, sy):
    x = np.asarray(x, dtype=np.float32)
    d = np.asarray(d, dtype=np.float32)
    W = np.asarray(W, dtype=np.float32)
    sx = np.asarray(sx, dtype=np.int32)
    sy = np.asarray(sy, dtype=np.int32)

    # Host-side shard prep, per the sharding strategy: gather the K station
    # feature vectors once (replicated to all cores), pre-transpose the
    # station features and each core's d shard to contraction-major (with the
    # 8-way row interleave the store DMA layout expects), and quantize the
    # wire tensors to fp16.
    station_t = np.ascontiguousarray(x[sx, sy].T, dtype=np.float16)
    w16 = W.astype(np.float16)

    nc = _build_program()

    nb = ROWS // BLK
    in_maps = []
    for c in range(NCORES):
        d_sh = d[c * HS : (c + 1) * HS].reshape(ROWS, K)
        # [blk, p, q, k] -> [k, blk, q, p]:
        #   d_t[k, blk*1024 + q*128 + p] = d_sh[blk*1024 + 8p + q, k]
        d_t = np.ascontiguousarray(
            d_sh.reshape(nb, 128, 8, K).transpose(3, 0, 2, 1),
            dtype=np.float16,
        ).reshape(K, ROWS)
        in_maps.append(
            {
                "d_t": d_t,
                "station_t": station_t,
                "w_mat": w16,
            }
        )

    res = run_bass_kernel_spmd(nc, in_maps, list(range(NCORES)))
    global last_results
    last_results = res
    out = np.concatenate(
        [
            r["out_shard"].astype(np.float32).reshape(HS, WG, F)
            for r in res.results
        ],
        axis=0,
    )
    return out


if __name__ == "__main__":
    rng = np.random.default_rng(0)
    x = rng.standard_normal((H, WG, F), dtype=np.float32)
    d = rng.random((H, WG, K), dtype=np.float32)
    W = rng.standard_normal((K, F), dtype=np.float32) / np.sqrt(F)
    sx = rng.integers(0, H, size=(K,)).astype(np.int32)
    sy = rng.integers(0, WG, size=(K,)).astype(np.int32)
    out = kernel(x, d, W, sx, sy)
    y = x[sx, sy].astype(np.float64) @ W.astype(np.float64)
    exp = d.reshape(-1, K).astype(np.float64) @ y
    exp = exp.reshape(H, WG, F)
    err = np.linalg.norm(out - exp) / np.linalg.norm(exp)
    print("rel err:", err)


# revision 7
# speedup vs baseline: 1.2780x; 1.0129x over previous
"""Trainium2 Bass kernel for nn_ContextualViewModel_48833778155979.

Computation (see reference):
    station_feats = x[sx, sy]            # (K, F) gather -- on host (the
                                         # sharding hint says to replicate it)
    y = station_feats @ W                # (K, F) tiny matmul -- on device
    res[h, w, :] = sum_k d[h, w, k] * y[k, :]   # big (H*W, K) @ (K, F) matmul

Sharding: H axis split across 8 cores (48 rows each -> 18432 grid cells/core).
Per core the big matmul is (18432, 256) @ (256, 256).

The kernel is DMA-roofline bound (HBM ~358 GB/s/core): mandatory traffic
is d in + out, moved as fp16 (9 + 9 MiB ~= 53 us; fp32 would be ~105 us).
(uint8 input was tried and reverted: GpSimd/DVE convert u8->fp16 at only
~0.3 elem/lane/cycle, costing more than the DMA it saves.)

  - d is laid out k-major during host-side shard prep, with an 8-way row
    interleave inside every 1024-row block (row blk*1024 + 8p + q stored
    at column blk*1024 + q*128 + p). The 128x128 stationary chunks DMA
    straight into SBUF (no PE transposes), input bursts are 4 KiB
    contiguous per partition, and each output partition owns 8
    consecutive DRAM rows so every 1024-row block stores as one DMA with
    a single contiguous 4 KiB burst per partition.
  - y (fp16, k-major) is the moving operand (256 wide); PSUM accumulates
    the two 128-wide k chunks in fp32. One PSUM bank = two 128-row
    output subtiles, drained by a single 512-elem cast to fp16.
  - PSUM drains are split 2/2 between DVE and ScalarE (PSUM has one DVE
    read port, so a single engine would be on the critical path);
    store-DMA descriptor gen lives on ScalarE, load-side on SyncE, and
    the constants load on ScalarE so the head overlaps.
  - 8 junk warmup matmuls run while the first d slab streams in, lifting
    the PE HAM clock throttle (1.2 -> 2.4 GHz) before the real work.

PE per core: 288 matmuls x 256 moving rows ~= 31 us warm, hidden under
~53 us of DMA. Accuracy: fp16 wire quantization, fp32 accumulation;
end-to-end rel err ~5e-4 (gate 1e-2).
"""

import sys

sys.path.insert(0, "/opt/trn_rl_repo")

from contextlib import ExitStack

import numpy as np

import concourse.bacc as bacc
import concourse.mybir as mybir
import concourse.tile as tile
from concourse.bass_utils import run_bass_kernel_spmd

H, WG, F = 384, 384, 256
K = 256
NCORES = 8
HS = H // NCORES          # 48 grid rows per core
ROWS = HS * WG            # 18432 cells per core
SLAB = 2048               # rows per input DMA slab (1 MiB fp16)
NSLAB = ROWS // SLAB      # 9
BLK = 1024                # rows per output DMA block / interleave group

F16 = mybir.dt.float16
F32 = mybir.dt.float32

_cache: dict = {}
last_results = None  # BassKernelResults of the most recent kernel() call


def _build_program():
    key = "nc"
    if key in _cache:
        return _cache[key]

    nc = bacc.Bacc(
        "TRN2", target_bir_lowering=False, debug=False, num_devices=NCORES
    )

    # d_t: per-core shard of d, k-major with 8-way row interleave (see
    # module docstring):
    #   d_t[k, blk*1024 + q*128 + p] = d_shard[blk*1024 + 8p + q, k]
    dt_ext = nc.dram_tensor("d_t", [K, ROWS], F16, kind="ExternalInput").ap()
    # station_t: gathered station features, transposed to (F_contract, K)
    stT_ext = nc.dram_tensor("station_t", [F, K], F16, kind="ExternalInput").ap()
    w_ext = nc.dram_tensor("w_mat", [F, F], F16, kind="ExternalInput").ap()
    out_ext = nc.dram_tensor("out_shard", [ROWS, F], F16, kind="ExternalOutput").ap()

    with tile.TileContext(nc) as tc, ExitStack() as ctx:
        const = ctx.enter_context(tc.tile_pool(name="const", bufs=1))
        dpool = ctx.enter_context(tc.tile_pool(name="din", bufs=5))
        opool = ctx.enter_context(tc.tile_pool(name="dout", bufs=6))
        mpsum = ctx.enter_context(tc.tile_pool(name="mpsum", bufs=6, space="PSUM"))
        ypsum = ctx.enter_context(tc.tile_pool(name="ypsum", bufs=1, space="PSUM"))
        wpsum = ctx.enter_context(tc.tile_pool(name="wpsum", bufs=1, space="PSUM"))

        # --- constants (first on the sync queue: tiny, so the PE warmup and
        # y matmuls start while the first d slab is still streaming) --------
        stT = const.tile([128, 2, K], F16)
        nc.sync.dma_start(
            stT[:, :, :], stT_ext.rearrange("(cc cp) k -> cp cc k", cc=2)
        )
        w_sb = const.tile([128, 2, F], F16)
        nc.sync.dma_start(
            w_sb[:, :, :], w_ext.rearrange("(cc cp) f -> cp cc f", cc=2)
        )

        # --- PE warmup -----------------------------------------------------
        # ~3.5 us of junk matmuls (result never read) while the first d slab
        # streams in: the HAM clock gate needs ~3.4 us of sustained PE
        # activity to lift the idle throttle (1.2 GHz -> 2.4 GHz).
        warm = wpsum.tile([128, 2, F], F32, tag="warm")
        for _ in range(8):
            nc.tensor.matmul(
                warm[:, :, :],
                stT[:, 0, 0:128],
                w_sb[:, :, :],
                start=True,
                stop=True,
            )

        # --- y = station_feats @ W, k-major in SBUF as fp16 ----------------
        # yps is one full PSUM bank; each 128-wide k chunk is its own
        # accumulation group in one half of the bank.
        y_sb = const.tile([128, 2, F], F16)
        yps = ypsum.tile([128, 2, F], F32, tag="yps")
        for kc in range(2):
            for cc in range(2):
                nc.tensor.matmul(
                    yps[:, kc, :],
                    stT[:, cc, kc * 128 : (kc + 1) * 128],
                    w_sb[:, cc, :],
                    start=(cc == 0),
                    stop=(cc == 1),
                )
        nc.vector.tensor_copy(y_sb[:, :, :], yps[:, :, :])

        # --- main loop: out = d @ y ---------------------------------------
        for s in range(NSLAB):
            din = dpool.tile([128, 2, SLAB], F16, tag="din")
            nc.sync.dma_start(
                din[:, :, :],
                dt_ext[:, s * SLAB : (s + 1) * SLAB].rearrange(
                    "(kc kp) r -> kp kc r", kc=2
                ),
            )
            for b in range(SLAB // BLK):
                # dout dims: [p, pr, qq, f] -- DRAM row = 8p + 2*pr + qq
                dout = opool.tile([128, 4, 2, F], F16, tag="dout")
                for pr in range(4):
                    po = mpsum.tile([128, 2, F], F32, tag="po")
                    for qq in range(2):
                        c0 = b * BLK + (pr * 2 + qq) * 128
                        for kc in range(2):
                            nc.tensor.matmul(
                                po[:, qq, :],
                                din[:, kc, c0 : c0 + 128],
                                y_sb[:, kc, :],
                                start=(kc == 0),
                                stop=(kc == 1),
                            )
                    if pr < 2:
                        nc.vector.tensor_copy(dout[:, pr, :, :], po[:, :, :])
                    else:
                        nc.scalar.copy(dout[:, pr, :, :], po[:, :, :])
                nc.scalar.dma_start(
                    out_ext[
                        (s * 2 + b) * BLK : (s * 2 + b + 1) * BLK, :
                    ].rearrange("(p pr qq) f -> p pr qq f", p=128, pr=4),
                    dout[:, :, :, :],
                )

    nc.compile()
    _cache[key] = nc
    return nc


def kernel(x, d, W, sx, sy):
    x = np.asarray(x, dtype=np.float32)
    d = np.asarray(d, dtype=np.float32)
    W = np.asarray(W, dtype=np.float32)
    sx = np.asarray(sx, dtype=np.int32)
    sy = np.asarray(sy, dtype=np.int32)

    # Host-side shard prep, per the sharding strategy: gather the K station
    # feature vectors once (replicated to all cores), pre-transpose the
    # station features and each core's d shard to contraction-major (with the
    # 8-way row interleave the store DMA layout expects), and quantize the
    # wire tensors to fp16.
    station_t = np.ascontiguousarray(x[sx, sy].T, dtype=np.float16)
    w16 = W.astype(np.float16)

    nc = _build_program()

    nb = ROWS // BLK
    in_maps = []
    for c in range(NCORES):
        d_sh = d[c * HS : (c + 1) * HS].reshape(ROWS, K)
        # [blk, p, q, k] -> [k, blk, q, p]:
        #   d_t[k, blk*1024 + q*128 + p] = d_sh[blk*1024 + 8p + q, k]
        d_t = np.ascontiguousarray(
            d_sh.reshape(nb, 128, 8, K).transpose(3, 0, 2, 1),
            dtype=np.float16,
        ).reshape(K, ROWS)
        in_maps.append(
            {
                "d_t": d_t,
                "station_t": station_t,
                "w_mat": w16,
            }
        )

    res = run_bass_kernel_spmd(nc, in_maps, list(range(NCORES)))
    global last_results
    last_results = res
    out = np.concatenate(
        [
            r["out_shard"].astype(np.float32).reshape(HS, WG, F)
            for r in res.results
        ],
        axis=0,
    )
    return out


if __name__ == "__main__":
    rng = np.random.default_rng(0)
    x = rng.standard_normal((H, WG, F), dtype=np.float32)
    d = rng.random((H, WG, K), dtype=np.float32)
    W = rng.standard_normal((K, F), dtype=np.float32) / np.sqrt(F)
    sx = rng.integers(0, H, size=(K,)).astype(np.int32)
    sy = rng.integers(0, WG, size=(K,)).astype(np.int32)
    out = kernel(x, d, W, sx, sy)
    y = x[sx, sy].astype(np.float64) @ W.astype(np.float64)
    exp = d.reshape(-1, K).astype(np.float64) @ y
    exp = exp.reshape(H, WG, F)
    err = np.linalg.norm(out - exp) / np.linalg.norm(exp)
    print("rel err:", err)
